# revision 1
# baseline (speedup 1.0000x reference)
"""Trainium2 Bass kernel for a 2-layer LSTM (batch 8192, seq 128, in 32, hidden 64)
with a final linear head producing one logit per batch element.

Strategy: pure data parallel over 8 NeuronCores (1024 batch each), weights
replicated.  The input projection is folded into the recurrent step (no
[B,T,4H] materialization) so HBM traffic is ~one read of x.

Per core the 1024 batch is split into 2 independent pipeline chains x 2
partition-halves of 256 columns.  Layout ("gate-pure"): per gate the
pre-activations live in PSUM as [128, 256] = [gate(half A); gate(half B)],
produced by one block-diagonal matmul (stationary diag(W_G, W_G), both halves
stacked along the contraction dim).  All ScalarE/VectorE elementwise ops then
run with 128 partitions busy.  Matmuls use float32r (full-rate fp32 mode).
Layer-0 biases ride ones-rows in x; layer-1 biases are per-partition bias APs
with gates grouped by equal bias vectors.
"""

import numpy as np

INPUT = 32
HIDDEN = 64
BATCH = 8192
SEQ = 128
NCORES = 8
BCORE = BATCH // NCORES      # 1024
NCH = 2                      # pipeline chains per core
BHC = BCORE // (2 * NCH)     # 256 columns per chain (x2 partition halves)
D1 = INPUT + 1               # x rows + ones row

_CACHE = {}


def _build_module(b1_groups):
    """b1_groups: list of (gate_idx_list, has_bias) covering device gates 0..3
    (device gate order [i, f, o, g]); gates in one group share a bias vector
    and must be column-adjacent (psum block order is [i, o, f, g])."""
    import concourse.bacc as bacc
    import concourse.mybir as mybir
    import concourse.tile as tile

    F32 = mybir.dt.float32
    F32R = mybir.dt.float32r
    AF = mybir.ActivationFunctionType
    MUL = mybir.AluOpType.mult
    ADD = mybir.AluOpType.add

    nc = bacc.Bacc()
    xT = nc.dram_tensor("xT", [SEQ, NCH, 2 * D1, BHC], F32R, kind="ExternalInput")
    wx0 = nc.dram_tensor("wx0", [4, 2 * D1, 128], F32R, kind="ExternalInput")
    wh0 = nc.dram_tensor("wh0", [4, 128, 128], F32R, kind="ExternalInput")
    w1a = nc.dram_tensor("w1a", [4, 128, 128], F32R, kind="ExternalInput")
    w1b = nc.dram_tensor("w1b", [4, 128, 128], F32R, kind="ExternalInput")
    b1d = nc.dram_tensor("b1d", [128, 4], F32, kind="ExternalInput")
    fcw = nc.dram_tensor("fcw", [128, 2], F32R, kind="ExternalInput")
    fcb = nc.dram_tensor("fcb", [2, 1], F32, kind="ExternalInput")
    out = nc.dram_tensor("out", [NCH, 2, BHC], F32, kind="ExternalOutput")

    with tile.TileContext(nc) as tc:
        with (
            tc.tile_pool(name="wp", bufs=1) as wp,
            tc.tile_pool(name="sb", bufs=3) as sb,
            tc.tile_pool(name="ps", bufs=1, space="PSUM") as ps,
        ):
            twx = [wp.tile([2 * D1, 128], F32R, name=f"twx{g}", tag=f"twx{g}") for g in range(4)]
            twh = [wp.tile([128, 128], F32R, name=f"twh{g}", tag=f"twh{g}") for g in range(4)]
            t1a = [wp.tile([128, 128], F32R, name=f"t1a{g}", tag=f"t1a{g}") for g in range(4)]
            t1b = [wp.tile([128, 128], F32R, name=f"t1b{g}", tag=f"t1b{g}") for g in range(4)]
            tb1 = wp.tile([128, 4], F32, name="tb1")
            tfcw = wp.tile([128, 2], F32R, name="tfcw")
            tfcb = wp.tile([2, 1], F32, name="tfcb")
            for g in range(4):
                nc.sync.dma_start(twx[g][:, :], wx0[g, :, :])
                nc.sync.dma_start(twh[g][:, :], wh0[g, :, :])
                nc.sync.dma_start(t1a[g][:, :], w1a[g, :, :])
                nc.sync.dma_start(t1b[g][:, :], w1b[g, :, :])
            nc.sync.dma_start(tb1[:, :], b1d[:, :])
            nc.sync.dma_start(tfcw[:, :], fcw[:, :])
            nc.sync.dma_start(tfcb[:, :], fcb[:, :])

            h1p = [None] * NCH
            h2p = [None] * NCH
            c0p = [None] * NCH
            c1p = [None] * NCH
            h2_last = [None] * NCH

            for t in range(SEQ):
                for ch in range(NCH):
                    first = h1p[ch] is None
                    C = f"c{ch}_"

                    xt = sb.tile([2 * D1, BHC], F32R, name=f"{C}xt{t}", tag=f"{C}xt", bufs=4)
                    nc.sync.dma_start(xt[:, :], xT[t, ch, :, :])

                    # ---- layer 0: gates in PSUM col blocks [i, o, f, g] ----
                    P0 = ps.tile([128, 4 * BHC], F32, name=f"{C}P0_{t}", tag=f"{C}P0", bufs=1)
                    for g in range(4):
                        blk = slice(g * BHC, (g + 1) * BHC)
                        nc.tensor.matmul(P0[:, blk], twx[g][:, :], xt[:, :],
                                         start=True, stop=first)
                        if not first:
                            nc.tensor.matmul(P0[:, blk], twh[g][:, :], h1p[ch][:, :],
                                             start=False, stop=True)

                    # one sigmoid over all blocks [i,o,f,g'] (g scaled x2 in
                    # weights; tanh(g) = 2*sigma(2g)-1 folded into DVE below)
                    sio0 = sb.tile([128, 4 * BHC], F32, name=f"{C}sio0_{t}", tag=f"{C}sio0", bufs=3)
                    nc.scalar.activation(sio0[:, :], P0[:, :], AF.Sigmoid)

                    # ig_half = (sigma(2g) - 0.5) * sigma(i) = i*tanh(g)/2
                    ig0 = sb.tile([128, BHC], F32, name=f"{C}ig0_{t}", tag=f"{C}ig0", bufs=3)
                    nc.vector.scalar_tensor_tensor(
                        ig0[:, :], sio0[:, 3 * BHC:4 * BHC], 0.5, sio0[:, 0:BHC],
                        mybir.AluOpType.subtract, MUL)
                    if c0p[ch] is None:
                        c0 = sb.tile([128, BHC], F32, name=f"{C}c0_{t}", tag=f"{C}c0", bufs=3)
                        nc.vector.tensor_scalar_mul(c0[:, :], ig0[:, :], 2.0)
                    else:
                        fc0 = sb.tile([128, BHC], F32, name=f"{C}fc0_{t}", tag=f"{C}fc0", bufs=3)
                        nc.vector.tensor_tensor(fc0[:, :], sio0[:, 2 * BHC:3 * BHC], c0p[ch][:, :], MUL)
                        c0 = sb.tile([128, BHC], F32, name=f"{C}c0_{t}", tag=f"{C}c0", bufs=3)
                        nc.vector.scalar_tensor_tensor(
                            c0[:, :], ig0[:, :], 2.0, fc0[:, :], MUL, ADD)
                    th0 = sb.tile([128, BHC], F32, name=f"{C}th0_{t}", tag=f"{C}th0", bufs=3)
                    nc.scalar.activation(th0[:, :], c0[:, :], AF.Tanh)
                    h1 = sb.tile([128, BHC], F32R, name=f"{C}h1_{t}", tag=f"{C}h1", bufs=3)
                    nc.vector.tensor_tensor(h1[:, :], sio0[:, BHC:2 * BHC], th0[:, :], MUL)

                    # ---- layer 1 ----
                    P1 = ps.tile([128, 4 * BHC], F32, name=f"{C}P1_{t}", tag=f"{C}P1", bufs=1)
                    for g in range(4):
                        blk = slice(g * BHC, (g + 1) * BHC)
                        nc.tensor.matmul(P1[:, blk], t1a[g][:, :], h1[:, :],
                                         start=True, stop=first)
                        if not first:
                            nc.tensor.matmul(P1[:, blk], t1b[g][:, :], h2p[ch][:, :],
                                             start=False, stop=True)

                    # L1 activations: all-sigmoid (g scaled x2), grouped by
                    # equal bias vectors over col order [i, o, g, f].
                    ga = sb.tile([128, 4 * BHC], F32, name=f"{C}ga_{t}", tag=f"{C}ga", bufs=3)
                    for idxs, has_bias in b1_groups:
                        lo, hi = idxs[0], idxs[-1] + 1
                        nc.scalar.activation(
                            ga[:, lo * BHC:hi * BHC], P1[:, lo * BHC:hi * BHC],
                            AF.Sigmoid,
                            bias=tb1[:, lo:lo + 1] if has_bias else 0.0)

                    ig1 = sb.tile([128, BHC], F32, name=f"{C}ig1_{t}", tag=f"{C}ig1", bufs=3)
                    nc.vector.scalar_tensor_tensor(
                        ig1[:, :], ga[:, 2 * BHC:3 * BHC], 0.5, ga[:, 0:BHC],
                        mybir.AluOpType.subtract, MUL)
                    if c1p[ch] is None:
                        c1 = sb.tile([128, BHC], F32, name=f"{C}c1_{t}", tag=f"{C}c1", bufs=3)
                        nc.vector.tensor_scalar_mul(c1[:, :], ig1[:, :], 2.0)
                    else:
                        fc1 = sb.tile([128, BHC], F32, name=f"{C}fc1_{t}", tag=f"{C}fc1", bufs=3)
                        nc.vector.tensor_tensor(fc1[:, :], ga[:, 3 * BHC:4 * BHC], c1p[ch][:, :], MUL)
                        c1 = sb.tile([128, BHC], F32, name=f"{C}c1_{t}", tag=f"{C}c1", bufs=3)
                        nc.vector.scalar_tensor_tensor(
                            c1[:, :], ig1[:, :], 2.0, fc1[:, :], MUL, ADD)
                    th1 = sb.tile([128, BHC], F32, name=f"{C}th1_{t}", tag=f"{C}th1", bufs=3)
                    nc.scalar.activation(th1[:, :], c1[:, :], AF.Tanh)
                    h2 = sb.tile([128, BHC], F32R, name=f"{C}h2_{t}", tag=f"{C}h2", bufs=3)
                    nc.vector.tensor_tensor(h2[:, :], ga[:, BHC:2 * BHC], th1[:, :], MUL)

                    h1p[ch], h2p[ch], c0p[ch], c1p[ch] = h1, h2, c0, c1
                    if t == SEQ - 1:
                        h2_last[ch] = h2

            # ---- final linear head ----
            for ch in range(NCH):
                Pf = ps.tile([2, BHC], F32, name=f"Pf{ch}", tag=f"c{ch}_P0")
                nc.tensor.matmul(Pf[:, :], tfcw[:, :], h2_last[ch][:, :], start=True, stop=True)
                ob = sb.tile([2, BHC], F32, name=f"ob{ch}")
                nc.scalar.activation(ob[:, :], Pf[:, :], AF.Identity, bias=tfcb[:, 0:1])
                nc.sync.dma_start(out[ch, :, :], ob[:, :])

    nc.compile()
    return nc


def _bias_groups(b1dev):
    """Group all 4 device gates (all sigmoid; L1 col order [i, o, g, f]) into
    column-adjacent runs sharing an identical bias vector."""
    groups = []
    run = [0]
    for g in range(1, 4):
        if np.array_equal(b1dev[g], b1dev[run[0]]):
            run.append(g)
        else:
            groups.append(run)
            run = [g]
    groups.append(run)
    return [(idxs, bool(np.any(b1dev[idxs[0]]))) for idxs in groups]


def _prep_weights(w_ih0, w_hh0, b_ih0, b_hh0, w_ih1, w_hh1, b_ih1, b_hh1, fc_w, fc_b):
    """Host-side packing. Device gate order: [i, o, f, g] (PyTorch order i,f,g,o)."""
    H = HIDDEN
    GATES = [0, 3, 1, 2]        # L0 device order [i, o, f, g] (pytorch i,f,g,o)
    GATES1 = [0, 3, 2, 1]       # L1 device order [i, o, g, f]
    b0 = (b_ih0 + b_hh0).reshape(4, H)
    b1 = (b_ih1 + b_hh1).reshape(4, H)
    wi0 = w_ih0.reshape(4, H, INPUT)
    wh0_ = w_hh0.reshape(4, H, H)
    wi1 = w_ih1.reshape(4, H, H)
    wh1_ = w_hh1.reshape(4, H, H)

    wx0 = np.zeros((4, 2 * D1, 128), np.float32)
    wh0 = np.zeros((4, 128, 128), np.float32)
    w1a = np.zeros((4, 128, 128), np.float32)
    w1b = np.zeros((4, 128, 128), np.float32)
    b1dev = np.zeros((4, H), np.float32)
    b1d = np.zeros((128, 4), np.float32)
    for k, gi in enumerate(GATES):
        sc = 2.0 if gi == 2 else 1.0      # pytorch gate 2 = g: pre-scale x2
        wt = sc * wi0[gi].T               # [INPUT, H]
        wx0[k, :INPUT, 0:H] = wt
        wx0[k, INPUT, 0:H] = sc * b0[gi]
        wx0[k, D1:D1 + INPUT, H:2 * H] = wt
        wx0[k, D1 + INPUT, H:2 * H] = sc * b0[gi]
        wh0[k, 0:H, 0:H] = sc * wh0_[gi].T
        wh0[k, H:2 * H, H:2 * H] = sc * wh0_[gi].T
    for k, gi in enumerate(GATES1):
        sc = 2.0 if gi == 2 else 1.0
        w1a[k, 0:H, 0:H] = sc * wi1[gi].T
        w1a[k, H:2 * H, H:2 * H] = sc * wi1[gi].T
        w1b[k, 0:H, 0:H] = sc * wh1_[gi].T
        w1b[k, H:2 * H, H:2 * H] = sc * wh1_[gi].T
        b1dev[k] = sc * b1[gi]
        b1d[0:H, k] = sc * b1[gi]
        b1d[H:2 * H, k] = sc * b1[gi]

    fcw = np.zeros((128, 2), np.float32)
    fcw[0:H, 0] = fc_w[0]
    fcw[H:2 * H, 1] = fc_w[0]
    fcb = np.full((2, 1), np.float32(fc_b[0]), np.float32)
    return wx0, wh0, w1a, w1b, b1dev, b1d, fcw, fcb


def run_full(x, w_ih0, w_hh0, b_ih0, b_hh0, w_ih1, w_hh1, b_ih1, b_hh1, fc_w, fc_b,
             trace=False):
    """Run the full problem on 8 cores; returns (output [BATCH], BassKernelResults)."""
    from concourse.bass_utils import run_bass_kernel_spmd

    x = np.asarray(x, np.float32)
    args = [np.asarray(a, np.float32) for a in
            (w_ih0, w_hh0, b_ih0, b_hh0, w_ih1, w_hh1, b_ih1, b_hh1, fc_w, fc_b)]
    wx0, wh0, w1a, w1b, b1dev, b1d, fcw, fcb = _prep_weights(*args)

    groups = _bias_groups(b1dev)
    key = tuple((tuple(i), b) for i, b in groups)
    if key not in _CACHE:
        _CACHE[key] = _build_module(groups)
    nc = _CACHE[key]

    in_maps = []
    for c in range(NCORES):
        xs = x[c * BCORE:(c + 1) * BCORE]                  # [BCORE, SEQ, INPUT]
        xT = np.empty((SEQ, NCH, 2 * D1, BHC), np.float32)
        for ch in range(NCH):
            a0 = ch * BHC
            b0_ = BCORE // 2 + ch * BHC
            xT[:, ch, :INPUT, :] = xs[a0:a0 + BHC].transpose(1, 2, 0)
            xT[:, ch, INPUT, :] = 1.0
            xT[:, ch, D1:D1 + INPUT, :] = xs[b0_:b0_ + BHC].transpose(1, 2, 0)
            xT[:, ch, D1 + INPUT, :] = 1.0
        in_maps.append({
            "xT": xT, "wx0": wx0, "wh0": wh0, "w1a": w1a, "w1b": w1b,
            "b1d": b1d, "fcw": fcw, "fcb": fcb,
        })

    res = run_bass_kernel_spmd(nc, in_maps, core_ids=list(range(NCORES)), trace=trace)
    outs = []
    for r in res.results:
        o = r["out"]                        # [NCH, 2, BHC]: (chain, half, col)
        # per-core batch order: [ch0 halfA, ch1 halfA, ch0 halfB, ch1 halfB]
        outs.append(o.transpose(1, 0, 2).reshape(BCORE))
    return np.concatenate(outs, axis=0).astype(np.float32), res


def kernel(x, w_ih0, w_hh0, b_ih0, b_hh0, w_ih1, w_hh1, b_ih1, b_hh1, fc_w, fc_b):
    out, _ = run_full(x, w_ih0, w_hh0, b_ih0, b_hh0,
                      w_ih1, w_hh1, b_ih1, b_hh1, fc_w, fc_b)
    return out

